# revision 1
# baseline (speedup 1.0000x reference)
"""DrBCNet GNN message-passing kernel for 8 Trainium2 NeuronCores.

Strategy (dst-sharded graph parallel):
  - Nodes are globally degree-sorted and dealt round-robin to the 8 cores
    (3750 each); each core owns its dst nodes, GRU/norm/decoder are node-local.
  - Per layer: the bf16 h-table (all-gathered each layer) stays SBUF-resident;
    TensorE accumulates aggT[feat, dst] = sum_blocks table_block.T @ A_block
    in PSUM, streaming the per-core bf16 adjacency A [table_rows, npc] from
    HBM as fp8 (exact 0/1/2 counts; memory-roofline bound). The bank range is
    processed in two halves so the first half's GRU/l2norm (ACT/DVE) overlaps
    the second half's matmuls. PE transposes h back to row layout which is
    AllGather'd (bf16) into the next layer's table.
"""

import functools
import os

import numpy as np

CORES = 8
H = 128
L = 5
BANK = 512  # fp32 PSUM bank width
NORM_EPS_SQ = 1e-24


# ---------------------------------------------------------------- host planning
def _plan(edge_src, edge_dst, n_nodes):
    npc = n_nodes // CORES
    npc_pad = ((npc + 127) // 128) * 128
    ntiles = npc_pad // 128

    # global degree sort, dealt round-robin to cores: core = rank % 8,
    # pos = rank // 8. Interleaved sorted sequences keep the per-position
    # cumulative degree nearly identical across cores (bounded drift), so
    # shared per-chunk PSUM column windows stay narrow.
    deg = np.bincount(edge_dst, minlength=n_nodes)
    gorder = np.argsort(-deg, kind="stable")  # rank -> node
    gpos = np.empty(n_nodes, np.int64)
    gpos[gorder] = np.arange(n_nodes)
    owner = gpos % CORES
    pos = gpos // CORES
    order_per_core = [gorder[r::CORES] for r in range(CORES)]  # pos -> node id
    core_edges = [np.nonzero(owner[edge_dst] == r)[0] for r in range(CORES)]

    tpos = owner * npc_pad + pos  # node -> table row
    n_banks = (npc + BANK - 1) // BANK
    import ml_dtypes

    n_tbl = CORES * npc_pad
    A = []
    for r in range(CORES):
        eidx = core_edges[r]
        Af = np.zeros((n_tbl, npc), np.float32)
        np.add.at(Af, (tpos[edge_src[eidx]], pos[edge_dst[eidx]]), 1.0)
        A.append(Af.astype(ml_dtypes.float8_e4m3fn))
    return dict(
        npc=npc,
        npc_pad=npc_pad,
        ntiles=ntiles,
        n_banks=n_banks,
        n_tbl=n_tbl,
        A=A,
        order_per_core=order_per_core,
        tpos=tpos,
    )


# ---------------------------------------------------------------- bass program
def _build(meta):
    import concourse.bacc as bacc
    import concourse.mybir as mybir
    import concourse.tile as tile
    from concourse.masks import make_identity

    npc = meta["npc"]
    npc_pad = meta["npc_pad"]
    ntiles = meta["ntiles"]
    n_banks = meta["n_banks"]
    n_tbl = meta["n_tbl"]
    n_blk = n_tbl // 128
    f32 = mybir.dt.float32
    bf16 = mybir.dt.bfloat16
    AF = mybir.ActivationFunctionType
    OP = mybir.AluOpType

    nc = bacc.Bacc(
        "TRN2", target_bir_lowering=False, debug=False, num_devices=CORES
    )

    # I/O
    xT_d = nc.dram_tensor("xT", [3, npc], f32, kind="ExternalInput")
    A_d = nc.dram_tensor("A", [n_tbl, npc], mybir.dt.float8e4, kind="ExternalInput")
    w1T_d = nc.dram_tensor("w1T", [3, 128], f32, kind="ExternalInput")
    b1_d = nc.dram_tensor("b1", [128, 1], f32, kind="ExternalInput")
    wihT_d = nc.dram_tensor("wihT", [128, 3 * H], f32, kind="ExternalInput")
    whhT_d = nc.dram_tensor("whhT", [128, 3 * H], f32, kind="ExternalInput")
    bih_d = nc.dram_tensor("bih", [1, 3 * H], f32, kind="ExternalInput")
    bhh_d = nc.dram_tensor("bhh", [1, 3 * H], f32, kind="ExternalInput")
    w2T_d = nc.dram_tensor("w2T", [128, 128], f32, kind="ExternalInput")
    b2_d = nc.dram_tensor("b2", [1, 128], f32, kind="ExternalInput")
    out_d = nc.dram_tensor("out", [npc_pad, 128], f32, kind="ExternalOutput")

    ag_in = [nc.dram_tensor(f"agin{l}", [npc_pad, 128], bf16) for l in range(L)]
    tables = [
        nc.dram_tensor(f"table{l}", [CORES * npc_pad, 128], bf16, addr_space="Shared")
        for l in range(L)
    ]
    groups = [list(range(CORES))]

    banks = [(b * BANK, min(BANK, npc - b * BANK)) for b in range(n_banks)]

    with tile.TileContext(nc) as tc:
        import contextlib

        stack = contextlib.ExitStack()
        per = stack.enter_context(tc.tile_pool(name="per", bufs=1))

        def _T(tc, shape, dtype, name=None):
            return per.tile(shape, dtype, name=name, tag=name)

        # persistent tiles
        table_sb = _T(tc, [128, n_blk, 128], bf16, name="table_sb")
        xT_sb = _T(tc, [3, npc], f32, name="xT_sb")
        hT = _T(tc, [128, npc], f32, name="hT")
        hmaxT = _T(tc, [128, npc], f32, name="hmaxT")
        aggT = _T(tc, [128, npc], f32, name="aggT")
        w1T_sb = _T(tc, [3, 128], f32, name="w1T_sb")
        b1_sb = _T(tc, [128, 1], f32, name="b1_sb")
        wihT_sb = _T(tc, [128, 3 * H], f32, name="wihT_sb")
        whhT_sb = _T(tc, [128, 3 * H], f32, name="whhT_sb")
        bih_sb = _T(tc, [1, 3 * H], f32, name="bih_sb")
        bhh_sb = _T(tc, [1, 3 * H], f32, name="bhh_sb")
        w2T_sb = _T(tc, [128, 128], f32, name="w2T_sb")
        b2_sb = _T(tc, [1, 128], f32, name="b2_sb")
        ones_col = _T(tc, [128, 1], f32, name="ones_col")
        ones_row = _T(tc, [1, BANK], f32, name="ones_row")
        onesk1 = _T(tc, [1, 128], f32, name="onesk1")
        ident = _T(tc, [128, 128], f32, name="ident")
        eps_sb = _T(tc, [1, 1], f32, name="eps_sb")

        xpool = stack.enter_context(tc.tile_pool(name="xpool", bufs=2))
        apool = stack.enter_context(tc.tile_pool(name="apool", bufs=2))
        gpool = stack.enter_context(tc.tile_pool(name="gpool", bufs=2))
        tpool = stack.enter_context(tc.tile_pool(name="tpool", bufs=1))
        ps = stack.enter_context(tc.tile_pool(name="ps", bufs=8, space="PSUM"))

        # input loads
        nc.sync.dma_start(out=xT_sb[:], in_=xT_d[:])
        nc.sync.dma_start(out=w1T_sb[:], in_=w1T_d[:])
        nc.sync.dma_start(out=b1_sb[:], in_=b1_d[:])
        nc.sync.dma_start(out=wihT_sb[:], in_=wihT_d[:])
        nc.sync.dma_start(out=whhT_sb[:], in_=whhT_d[:])
        nc.sync.dma_start(out=bih_sb[:], in_=bih_d[:])
        nc.sync.dma_start(out=bhh_sb[:], in_=bhh_d[:])
        nc.sync.dma_start(out=w2T_sb[:], in_=w2T_d[:])
        nc.sync.dma_start(out=b2_sb[:], in_=b2_d[:])
        nc.vector.memset(eps_sb[:], NORM_EPS_SQ)
        nc.vector.memset(ones_col[:], 1.0)
        nc.vector.memset(ones_row[:], 1.0)
        nc.vector.memset(onesk1[:], 1.0)
        make_identity(nc, ident[:])

        def norm_strip(b, s0, w):
            """hT[:, s0:s0+w] /= (sqrt(sum_f hT^2) + eps), per node column."""
            sq = tpool.tile([128, BANK], f32, tag="sq")
            nc.vector.tensor_tensor(
                out=sq[:, :w], in0=hT[:, s0 : s0 + w], in1=hT[:, s0 : s0 + w],
                op=OP.mult,
            )
            ns_ps = ps.tile([1, BANK], f32, tag="ps", name=f"ns{b}")
            nc.tensor.matmul(
                out=ns_ps[:1, :w], lhsT=ones_col[:], rhs=sq[:, :w],
                start=True, stop=True,
            )
            srt = tpool.tile([1, BANK], f32, tag="srt")
            nc.scalar.activation(
                out=srt[:1, :w], in_=ns_ps[:1, :w], func=AF.Sqrt, bias=eps_sb[:1, :1]
            )
            inv_t = tpool.tile([1, BANK], f32, tag="inv_t")
            nc.vector.reciprocal(out=inv_t[:1, :w], in_=srt[:1, :w])
            bc_ps = ps.tile([128, BANK], f32, tag="ps", name=f"bc{b}")
            nc.tensor.matmul(
                out=bc_ps[:, :w], lhsT=onesk1[:1, :], rhs=inv_t[:1, :w],
                start=True, stop=True,
            )
            nc.vector.tensor_tensor(
                out=hT[:, s0 : s0 + w], in0=hT[:, s0 : s0 + w], in1=bc_ps[:, :w],
                op=OP.mult,
            )

        def store_and_allgather(l):
            rows = xpool.tile([128, ntiles, 128], bf16, tag="xbuf", name=f"rows{l}")
            if npc - (ntiles - 1) * 128 < 128:
                nc.vector.memset(rows[:, ntiles - 1, :], 0.0)
            for t in range(ntiles):
                wt = min(128, npc - t * 128)
                if wt <= 0:
                    break
                tp_ps = ps.tile([128, 128], f32, tag="ps", name=f"tp{l}_{t}")
                nc.tensor.transpose(
                    out=tp_ps[:wt, :], in_=hT[:, t * 128 : t * 128 + wt],
                    identity=ident[:],
                )
                nc.scalar.activation(
                    out=rows[:wt, t, :], in_=tp_ps[:wt, :], func=AF.Copy
                )
            dst = ag_in[l].ap().rearrange("(c p) f -> p c f", p=128)
            nc.sync.dma_start(out=dst, in_=rows[:])
            nc.gpsimd.collective_compute(
                "AllGather",
                OP.bypass,
                replica_groups=groups,
                ins=[ag_in[l][:]],
                outs=[tables[l][:]],
            )

        # ---------------- encoder: hT = l2norm(relu(W1 @ x + b1)), hmax = hT
        for b, (s0, w) in enumerate(banks):
            h0_ps = ps.tile([128, BANK], f32, tag="ps", name=f"enc{b}")
            nc.tensor.matmul(
                out=h0_ps[:, :w], lhsT=w1T_sb[:], rhs=xT_sb[:, s0 : s0 + w],
                start=True, stop=True,
            )
            nc.scalar.activation(
                out=hT[:, s0 : s0 + w], in_=h0_ps[:, :w], func=AF.Relu,
                bias=b1_sb[:, :1],
            )
            norm_strip(b, s0, w)
            nc.vector.tensor_copy(out=hmaxT[:, s0 : s0 + w], in_=hT[:, s0 : s0 + w])
        store_and_allgather(0)

        # ---------------- message-passing layers
        for l in range(L):
            # full bf16 table -> SBUF as [128 rows-in-block, block, feat]
            nc.sync.dma_start(
                out=table_sb[:],
                in_=tables[l].ap().rearrange("(u p) f -> p u f", p=128),
            )
            nh = (n_banks + 1) // 2
            halves = [list(enumerate(banks))[:nh], list(enumerate(banks))[nh:]]
            for hi, hbanks in enumerate(halves):
                if not hbanks:
                    continue
                c0 = hbanks[0][1][0]
                c1 = hbanks[-1][1][0] + hbanks[-1][1][1]
                agg_ps = {}
                for b, (s0, w) in hbanks:
                    agg_ps[b] = ps.tile(
                        [128, BANK], f32, tag="ps", name=f"agg{l}_{b}"
                    )
                # batch G src-blocks per A DMA (~1 MB transfers -> full DMA bw)
                G = 4 if n_blk % 4 == 0 else (2 if n_blk % 2 == 0 else 1)
                for ug0 in range(0, n_blk, G):
                    a_sb = apool.tile(
                        [128, G, c1 - c0], mybir.dt.float8e4, tag="a_sb",
                        name=f"a{l}_{hi}_{ug0}",
                    )
                    nc.sync.dma_start(
                        out=a_sb[:],
                        in_=A_d[ug0 * 128 : (ug0 + G) * 128, c0:c1].rearrange(
                            "(g p) c -> p g c", p=128
                        ),
                    )
                    for g in range(G):
                        ug = ug0 + g
                        for b, (s0, w) in hbanks:
                            nc.tensor.matmul(
                                out=agg_ps[b][:, :w],
                                lhsT=table_sb[:, ug, :],
                                rhs=a_sb[:, g, s0 - c0 : s0 - c0 + w],
                                start=(ug == 0),
                                stop=(ug == n_blk - 1),
                            )
                for b, (s0, w) in hbanks:
                    apb = agg_ps[b]
                    # evacuate aggT strip, then GRU for this strip
                    nc.scalar.activation(
                        out=aggT[:, s0 : s0 + w], in_=apb[:, :w], func=AF.Copy
                    )

                    gis, ghs = [], []
                    for g in range(3):
                        gi_ps = ps.tile([128, BANK], f32, tag="ps", name=f"gi{l}{b}{g}")
                        nc.tensor.matmul(
                            out=gi_ps[:, :w], lhsT=bih_sb[:1, g * H : (g + 1) * H],
                            rhs=ones_row[:1, :w], start=True, stop=False,
                        )
                        nc.tensor.matmul(
                            out=gi_ps[:, :w], lhsT=wihT_sb[:, g * H : (g + 1) * H],
                            rhs=aggT[:, s0 : s0 + w], start=False, stop=True,
                        )
                        gi = gpool.tile([128, BANK], f32, tag=f"gi{g}")
                        nc.scalar.activation(out=gi[:, :w], in_=gi_ps[:, :w], func=AF.Copy)
                        gis.append(gi)
                        gh_ps = ps.tile([128, BANK], f32, tag="ps", name=f"gh{l}{b}{g}")
                        nc.tensor.matmul(
                            out=gh_ps[:, :w], lhsT=bhh_sb[:1, g * H : (g + 1) * H],
                            rhs=ones_row[:1, :w], start=True, stop=False,
                        )
                        nc.tensor.matmul(
                            out=gh_ps[:, :w], lhsT=whhT_sb[:, g * H : (g + 1) * H],
                            rhs=hT[:, s0 : s0 + w], start=False, stop=True,
                        )
                        gh = gpool.tile([128, BANK], f32, tag=f"gh{g}")
                        nc.scalar.activation(out=gh[:, :w], in_=gh_ps[:, :w], func=AF.Copy)
                        ghs.append(gh)

                    # r = sig(i_r + h_r); z = sig(i_z + h_z); n = tanh(i_n + r*h_n)
                    r_t = tpool.tile([128, BANK], f32, tag="r_t")
                    nc.vector.tensor_tensor(
                        out=r_t[:, :w], in0=gis[0][:, :w], in1=ghs[0][:, :w], op=OP.add
                    )
                    nc.scalar.activation(out=r_t[:, :w], in_=r_t[:, :w], func=AF.Sigmoid)
                    z_t = tpool.tile([128, BANK], f32, tag="z_t")
                    nc.vector.tensor_tensor(
                        out=z_t[:, :w], in0=gis[1][:, :w], in1=ghs[1][:, :w], op=OP.add
                    )
                    nc.scalar.activation(out=z_t[:, :w], in_=z_t[:, :w], func=AF.Sigmoid)
                    n_t = tpool.tile([128, BANK], f32, tag="n_t")
                    nc.vector.tensor_tensor(
                        out=n_t[:, :w], in0=r_t[:, :w], in1=ghs[2][:, :w], op=OP.mult
                    )
                    nc.vector.tensor_tensor(
                        out=n_t[:, :w], in0=n_t[:, :w], in1=gis[2][:, :w], op=OP.add
                    )
                    nc.scalar.activation(out=n_t[:, :w], in_=n_t[:, :w], func=AF.Tanh)
                    # h' = n + z * (h - n)
                    d_t = tpool.tile([128, BANK], f32, tag="d_t")
                    nc.vector.tensor_tensor(
                        out=d_t[:, :w], in0=hT[:, s0 : s0 + w], in1=n_t[:, :w],
                        op=OP.subtract,
                    )
                    nc.vector.tensor_tensor(
                        out=d_t[:, :w], in0=d_t[:, :w], in1=z_t[:, :w], op=OP.mult
                    )
                    nc.vector.tensor_tensor(
                        out=hT[:, s0 : s0 + w], in0=d_t[:, :w], in1=n_t[:, :w], op=OP.add
                    )
                    norm_strip(b, s0, w)
                    nc.vector.tensor_tensor(
                        out=hmaxT[:, s0 : s0 + w], in0=hmaxT[:, s0 : s0 + w],
                        in1=hT[:, s0 : s0 + w], op=OP.max,
                    )
            if l < L - 1:
                store_and_allgather(l + 1)

        # ---------------- decoder: out = hmax @ W2.T + b2 (row layout)
        for t in range(ntiles):
            wt = min(128, npc - t * 128)
            o_ps = ps.tile([128, 128], f32, tag="ps", name=f"dec{t}")
            nc.tensor.matmul(
                out=o_ps[:wt, :], lhsT=onesk1[:1, :wt], rhs=b2_sb[:1, :],
                start=True, stop=False,
            )
            nc.tensor.matmul(
                out=o_ps[:wt, :], lhsT=hmaxT[:, t * 128 : t * 128 + wt],
                rhs=w2T_sb[:], start=False, stop=True,
            )
            orow = tpool.tile([128, 128], f32, tag="orow")
            nc.scalar.activation(out=orow[:wt, :], in_=o_ps[:wt, :], func=AF.Copy)
            nc.sync.dma_start(
                out=out_d[t * 128 : t * 128 + wt, :], in_=orow[:wt, :]
            )
        stack.close()

    nc.compile()
    return nc


# ---------------------------------------------------------------- entry points
def _prep(inputs):
    x = np.asarray(inputs["x"], np.float32)
    edge_src = np.asarray(inputs["edge_src"], np.int64)
    edge_dst = np.asarray(inputs["edge_dst"], np.int64)
    n_nodes = x.shape[0]
    meta = _plan(edge_src, edge_dst, n_nodes)
    npc = meta["npc"]

    W1 = np.asarray(inputs["W1"], np.float32)
    b1 = np.asarray(inputs["b1"], np.float32)
    W_ih = np.asarray(inputs["W_ih"], np.float32)
    b_ih = np.asarray(inputs["b_ih"], np.float32)
    W_hh = np.asarray(inputs["W_hh"], np.float32)
    b_hh = np.asarray(inputs["b_hh"], np.float32)
    W2 = np.asarray(inputs["W2"], np.float32)
    b2 = np.asarray(inputs["b2"], np.float32)

    shared = dict(
        w1T=np.ascontiguousarray(W1.T),
        b1=np.ascontiguousarray(b1[:, None]),
        wihT=np.ascontiguousarray(W_ih.T),
        whhT=np.ascontiguousarray(W_hh.T),
        bih=np.ascontiguousarray(b_ih[None, :]),
        bhh=np.ascontiguousarray(b_hh[None, :]),
        w2T=np.ascontiguousarray(W2.T),
        b2=np.ascontiguousarray(b2[None, :]),
    )
    in_maps = []
    for r in range(CORES):
        xr = x[meta["order_per_core"][r]]
        in_maps.append(
            dict(
                xT=np.ascontiguousarray(xr.T),
                A=meta["A"][r],
                **shared,
            )
        )
    return meta, in_maps


def _assemble(meta, results, n_nodes):
    npc = meta["npc"]
    out = np.empty((n_nodes, 128), np.float32)
    for r in range(CORES):
        out[meta["order_per_core"][r]] = results[r]["out"][:npc]
    return out


@functools.lru_cache(maxsize=1)
def _get_compiled(key):
    # key is a hash of the planning inputs; real data passed via _PENDING
    meta, in_maps = _PENDING[key]
    nc = _build(meta)
    return nc, meta, in_maps


_PENDING = {}


def kernel(**inputs):
    x = np.asarray(inputs["x"])
    n_nodes = x.shape[0]
    meta, in_maps = _prep(inputs)
    key = hash(
        (
            n_nodes,
            np.asarray(inputs["edge_src"]).tobytes(),
            np.asarray(inputs["edge_dst"]).tobytes(),
        )
    )
    _PENDING[key] = (meta, in_maps)
    nc, meta, _ = _get_compiled(key)

    from concourse.bass_utils import run_bass_kernel_spmd

    trace = bool(int(os.environ.get("KERNEL_TRACE", "0")))
    res = run_bass_kernel_spmd(
        nc, in_maps, core_ids=list(range(CORES)), trace=trace
    )
    kernel.last_results = res
    return _assemble(meta, res.results, n_nodes)



# revision 28
# speedup vs baseline: 2.8227x; 2.8227x over previous
"""DrBCNet GNN message-passing kernel for 8 Trainium2 NeuronCores.

Strategy (src-owner graph parallel, sparse gather + one-hot scatter matmul):
  - Core r owns nodes [r*3750, (r+1)*3750): their features, GRU state, and
    their OUT-edges (~75K/core). Per layer:
      1. dma_gather (SWDGE, 896-idx pieces -- the runtime's descriptor ring
         caps ~64 descs/queue) fetches h[src] rows (bf16, 256B) from the
         core's own row-table in DRAM, in edge-slot order (slots grouped by
         global 64-wide dst tile, padded per tile to the max count across
         the 8 cores so all cores share one SPMD instruction stream).
      2. Scatter: per chunk segment, matmul Eh^T(lhsT bf16) x D(one-hot dst
         fp8 rhs, 64-col window) accumulates partial aggT[feat, dst] tiles
         in PSUM over all 480 global dst tiles; evacuated bf16 to DRAM.
      3. ReduceScatter(add, bf16) per dst-QUARTER sums the 8 partials and
         hands each core its own shard's aggT column-major; emission is
         software-pipelined (RS-Qk issued right after quarter k's stores;
         GRU of quarter k emitted during quarter k+1's scatter) so only the
         last quarter's RS+GRU tail is exposed.
      4. GRU gates as bf16 matmuls with biases folded into ACT evacuations;
         h state and l2norm in fp32; cross-layer max + decoder in bf16.
"""

import functools
import os

import numpy as np

CORES = 8
H = 128
L = 5
BANK = 512
NORM_EPS_SQ = 1e-24
TW = 64  # scatter dst-tile width (D window)
PIECE_CHUNKS = 7  # gather piece = 896 idxs (real SWDGE ring: ~64 descs/queue)
QUARTER_TILES = (40, 16, 4)  # 64-wide tiles per RS group (big overlap group + shrinking tail)


# ---------------------------------------------------------------- host planning
def _plan(edge_src, edge_dst, n_nodes):
    import ml_dtypes

    npc = n_nodes // CORES
    npc_pad = ((npc + 127) // 128) * 128
    nt = npc_pad // TW  # scatter tiles per shard (60)
    qb = np.concatenate(([0], np.cumsum(QUARTER_TILES)))

    owner = edge_src // npc
    q = edge_dst // npc
    pos = edge_dst % npc
    tau = pos // TW
    col = pos % TW

    # global tile order: quarter-major, then dst-owner, then tile
    gt_list = []
    for k in range(4):
        for qv in range(CORES):
            for tv in range(qb[k], qb[k + 1]):
                gt_list.append((qv, tv))
    n_gt = len(gt_list)
    rank_of = np.zeros((CORES, nt), np.int64)
    for i, (qv, tv) in enumerate(gt_list):
        rank_of[qv, tv] = i
    rank_e = rank_of[q, tau]

    cnt = np.zeros((CORES, n_gt), np.int64)
    np.add.at(cnt, (owner, rank_e), 1)
    S = cnt.max(axis=0)
    off = np.concatenate(([0], np.cumsum(S)))
    nslot_used = int(off[-1])
    nchunk = (nslot_used + 127) // 128
    nslot = nchunk * 128

    # segments: one 64-col D group per (tile, chunk) intersection; matmuls
    # always span partitions [0,128) (PE tile_position must be 0), rows
    # outside the segment's slot range are zero in D.
    segs = []  # per gt: list of (chunk, seg_id)
    seg_of_slot = np.full(nslot, -1, np.int64)
    nseg = 0
    for g in range(n_gt):
        s, e = int(off[g]), int(off[g + 1])
        lst = []
        if e > s:
            for c in range(s // 128, (e - 1) // 128 + 1):
                lo = max(s, c * 128)
                hi = min(e, (c + 1) * 128)
                seg_of_slot[lo:hi] = nseg
                lst.append((c, nseg))
                nseg += 1
        segs.append(lst)

    cores = []
    for r in range(CORES):
        eidx = np.nonzero(owner == r)[0]
        er = rank_e[eidx]
        order = np.argsort(er, kind="stable")
        eidx = eidx[order]
        er = er[order]
        within = np.arange(len(eidx)) - np.searchsorted(er, er)
        slot = off[er] + within
        idx_flat = np.zeros(nslot, np.int16)
        idx_flat[slot] = (edge_src[eidx] - r * npc).astype(np.int16)
        idx_w = np.zeros((128, nslot // 16), np.int16)
        jj = np.arange(nslot)
        for rep in range(8):
            idx_w[jj % 16 + 16 * rep, jj // 16] = idx_flat
        D = np.zeros((128, nseg * TW), np.float32)
        D[slot % 128, seg_of_slot[slot] * TW + col[eidx]] = 1.0
        D = D.astype(ml_dtypes.float8_e4m3fn)
        cores.append(dict(idx_w=idx_w, D=D))

    return dict(
        npc=npc,
        npc_pad=npc_pad,
        nt=nt,
        n_gt=n_gt,
        gt_list=gt_list,
        segs=segs,
        nseg=nseg,
        nchunk=nchunk,
        nslot=nslot,
        cores=cores,
    )


# ---------------------------------------------------------------- bass program
def _build(meta):
    import contextlib

    import concourse.bacc as bacc
    import concourse.mybir as mybir
    import concourse.tile as tile
    from concourse import library_config
    from concourse.masks import make_identity

    npc = meta["npc"]
    npc_pad = meta["npc_pad"]
    nt = meta["nt"]
    n_gt = meta["n_gt"]
    gt_list = meta["gt_list"]
    segs = meta["segs"]
    nseg = meta["nseg"]
    nchunk = meta["nchunk"]
    nslot = meta["nslot"]
    npieces = (nchunk + PIECE_CHUNKS - 1) // PIECE_CHUNKS

    f32 = mybir.dt.float32
    bf16 = mybir.dt.bfloat16
    fp8 = mybir.dt.float8e4
    i16 = mybir.dt.int16
    AF = mybir.ActivationFunctionType
    OP = mybir.AluOpType

    qb = [0]
    for t in QUARTER_TILES:
        qb.append(qb[-1] + t)
    qcols = [t * TW for t in QUARTER_TILES]  # 1024,1024,1024,768
    qcol0 = [b * TW for b in qb[:4]]
    n_gt_q = [CORES * t for t in QUARTER_TILES]
    qg0 = [0]
    for n in n_gt_q:
        qg0.append(qg0[-1] + n)
    nrt = npc_pad // 128  # 128-wide row tiles (transposes/decoder)
    qrt = [c // 128 for c in qcols]  # row tiles per quarter: 8,8,8,6
    qrt0 = [c // 128 for c in qcol0]

    nc = bacc.Bacc(
        "TRN2", target_bir_lowering=False, debug=False, num_devices=CORES
    )

    xT_d = nc.dram_tensor("xT", [3, npc_pad], f32, kind="ExternalInput")
    idx_d = nc.dram_tensor("idx", [128, nslot // 16], i16, kind="ExternalInput")
    D_d = nc.dram_tensor("D", [128, nseg * TW], fp8, kind="ExternalInput")
    w1T_d = nc.dram_tensor("w1T", [3, 128], f32, kind="ExternalInput")
    b1_d = nc.dram_tensor("b1", [128, 1], f32, kind="ExternalInput")
    wihT_d = nc.dram_tensor("wihT", [128, 3 * H], f32, kind="ExternalInput")
    whhT_d = nc.dram_tensor("whhT", [128, 3 * H], f32, kind="ExternalInput")
    brz_d = nc.dram_tensor("brz", [128, 2], f32, kind="ExternalInput")
    bin_d = nc.dram_tensor("bin", [128, 1], f32, kind="ExternalInput")
    bhn_d = nc.dram_tensor("bhn", [128, 1], f32, kind="ExternalInput")
    w2T_d = nc.dram_tensor("w2T", [128, 128], f32, kind="ExternalInput")
    b2_d = nc.dram_tensor("b2", [1, 128], f32, kind="ExternalInput")
    out_d = nc.dram_tensor("out", [npc_pad, 128], f32, kind="ExternalOutput")

    hrows = [nc.dram_tensor(f"hrows{i}", [npc_pad, 128], bf16) for i in range(2)]
    partQ = [
        [
            nc.dram_tensor(f"part{k}_{i}", [CORES * 128, qcols[k]], bf16)
            for i in range(2)
        ]
        for k in range(4)
    ]
    aggQ = [
        [nc.dram_tensor(f"agg{k}_{i}", [128, qcols[k]], bf16) for i in range(2)]
        for k in range(4)
    ]
    groups = [list(range(CORES))]

    with tile.TileContext(nc) as tc:
        stack = contextlib.ExitStack()
        per = stack.enter_context(tc.tile_pool(name="per", bufs=1))

        def _T(shape, dtype, name):
            return per.tile(shape, dtype, name=name, tag=name)

        idx_sb = _T([128, nslot // 16], i16, "idx_sb")
        D_sb = _T([128, nseg * TW], fp8, "D_sb")
        hT = _T([128, npc_pad], f32, "hT")
        hbf = _T([128, npc_pad], bf16, "hbf")
        hmax = _T([128, npc_pad], bf16, "hmax")
        aggbf = _T([128, npc_pad], bf16, "aggbf")
        w1T_sb = _T([3, 128], f32, "w1T_sb")
        b1_sb = _T([128, 1], f32, "b1_sb")
        wih_bf = _T([128, 3 * H], bf16, "wih_bf")
        whh_bf = _T([128, 3 * H], bf16, "whh_bf")
        brz_sb = _T([128, 2], f32, "brz_sb")
        bin_sb = _T([128, 1], f32, "bin_sb")
        bhn_sb = _T([128, 1], f32, "bhn_sb")
        w2_bf = _T([128, 128], bf16, "w2_bf")
        b2_bf = _T([1, 128], bf16, "b2_bf")
        ones_col = _T([128, 1], f32, "ones_col")
        onesk1 = _T([1, 128], f32, "onesk1")
        onesk1_bf = _T([1, 128], bf16, "onesk1_bf")
        ident = _T([128, 128], f32, "ident")
        eps_sb = _T([1, 1], f32, "eps_sb")

        ehpool = stack.enter_context(tc.tile_pool(name="ehpool", bufs=8))
        stpool = stack.enter_context(tc.tile_pool(name="stpool", bufs=2))
        gpool = stack.enter_context(tc.tile_pool(name="gpool", bufs=2))
        rpool = stack.enter_context(tc.tile_pool(name="rpool", bufs=2))
        rowpool = stack.enter_context(tc.tile_pool(name="rowpool", bufs=2))
        ldpool = stack.enter_context(tc.tile_pool(name="ldpool", bufs=1))
        decpool = stack.enter_context(tc.tile_pool(name="decpool", bufs=2))
        ps = stack.enter_context(tc.tile_pool(name="ps", bufs=1, space="PSUM"))
        psb = stack.enter_context(tc.tile_pool(name="psb", bufs=1, space="PSUM"))
        ps_sc = stack.enter_context(
            tc.tile_pool(name="ps_sc", bufs=3, space="PSUM")
        )

        # ---- loads
        nc.gpsimd.load_library(library_config.mlp)
        nc.sync.dma_start(out=idx_sb[:], in_=idx_d[:])
        nc.sync.dma_start(out=D_sb[:], in_=D_d[:])
        nc.sync.dma_start(out=w1T_sb[:], in_=w1T_d[:])
        nc.sync.dma_start(out=b1_sb[:], in_=b1_d[:])
        wihT_f = ldpool.tile([128, 3 * H], f32, tag="wihf")
        whhT_f = ldpool.tile([128, 3 * H], f32, tag="whhf")
        nc.sync.dma_start(out=wihT_f[:], in_=wihT_d[:])
        nc.sync.dma_start(out=whhT_f[:], in_=whhT_d[:])
        nc.vector.tensor_copy(out=wih_bf[:], in_=wihT_f[:])
        nc.vector.tensor_copy(out=whh_bf[:], in_=whhT_f[:])
        nc.sync.dma_start(out=brz_sb[:], in_=brz_d[:])
        nc.sync.dma_start(out=bin_sb[:], in_=bin_d[:])
        nc.sync.dma_start(out=bhn_sb[:], in_=bhn_d[:])
        w2T_f = ldpool.tile([128, 128], f32, tag="w2f")
        nc.sync.dma_start(out=w2T_f[:], in_=w2T_d[:])
        nc.vector.tensor_copy(out=w2_bf[:], in_=w2T_f[:])
        b2_f = ldpool.tile([1, 128], f32, tag="b2f")
        nc.sync.dma_start(out=b2_f[:], in_=b2_d[:])
        nc.vector.tensor_copy(out=b2_bf[:], in_=b2_f[:])
        nc.vector.memset(eps_sb[:], NORM_EPS_SQ)
        nc.vector.memset(ones_col[:], 1.0)
        nc.vector.memset(onesk1[:], 1.0)
        nc.vector.memset(onesk1_bf[:], 1.0)
        make_identity(nc, ident[:])

        def norm_strip(tag, s0, w):
            sq = rpool.tile([128, BANK], f32, tag="sq")
            nc.vector.tensor_tensor(
                out=sq[:, :w], in0=hT[:, s0 : s0 + w], in1=hT[:, s0 : s0 + w],
                op=OP.mult,
            )
            ns_ps = psb.tile([128, BANK], f32, tag="bcns", name=f"ns{tag}")
            nc.tensor.matmul(
                out=ns_ps[:1, :w], lhsT=ones_col[:], rhs=sq[:, :w],
                start=True, stop=True,
            )
            inv_t = rpool.tile([1, BANK], f32, tag="inv_t")
            nc.scalar.activation(
                out=inv_t[:1, :w], in_=ns_ps[:1, :w], func=AF.Sqrt,
                bias=eps_sb[:1, :1],
            )
            nc.vector.reciprocal(out=inv_t[:1, :w], in_=inv_t[:1, :w])
            bc_ps = psb.tile([128, BANK], f32, tag="bcns", name=f"bc{tag}")
            nc.tensor.matmul(
                out=bc_ps[:, :w], lhsT=onesk1[:1, :], rhs=inv_t[:1, :w],
                start=True, stop=True,
            )
            nc.vector.tensor_tensor(
                out=hT[:, s0 : s0 + w], in0=hT[:, s0 : s0 + w], in1=bc_ps[:, :w],
                op=OP.mult,
            )
            nc.scalar.activation(
                out=hbf[:, s0 : s0 + w], in_=hT[:, s0 : s0 + w], func=AF.Copy
            )
            nc.vector.tensor_tensor(
                out=hmax[:, s0 : s0 + w], in0=hmax[:, s0 : s0 + w],
                in1=hbf[:, s0 : s0 + w], op=OP.max,
            )

        def store_rows(l, k):
            """Transpose quarter k's h cols to rows, DMA into hrows[l%2]."""
            t0, tn = qrt0[k], qrt0[k] + qrt[k]
            rows = rowpool.tile([128, max(qrt), 128], bf16, tag="rows")
            for t in range(t0, tn):
                tp_ps = psb.tile([128, BANK], f32, tag="bcns", name=f"tp{l}_{t}")
                nc.tensor.transpose(
                    out=tp_ps[:, :128], in_=hT[:, t * 128 : (t + 1) * 128],
                    identity=ident[:],
                )
                if t % 2 == 0:
                    nc.scalar.activation(
                        out=rows[:, t - t0, :], in_=tp_ps[:, :128], func=AF.Copy
                    )
                else:
                    nc.vector.tensor_copy(
                        out=rows[:, t - t0, :], in_=tp_ps[:, :128]
                    )
            dst = hrows[l % 2][t0 * 128 : tn * 128, :].rearrange(
                "(u p) f -> p u f", p=128
            )
            nc.sync.dma_start(out=dst, in_=rows[:, : tn - t0, :])

        def gru_quarter(l, k):
            agg_d = aggQ[k][l % 2]
            s_base = qcol0[k]
            nc.sync.dma_start(
                out=aggbf[:, s_base : s_base + qcols[k]], in_=agg_d[:]
            )
            s0 = s_base
            while s0 < s_base + qcols[k]:
                w = min(BANK, s_base + qcols[k] - s0)
                tag = f"{l}_{s0}"
                gsb = []
                for g in range(2):
                    gp = ps.tile([128, BANK], f32, tag=f"g{g}", name=f"g{g}_{tag}")
                    nc.tensor.matmul(
                        out=gp[:, :w], lhsT=wih_bf[:, g * H : (g + 1) * H],
                        rhs=aggbf[:, s0 : s0 + w], start=True, stop=False,
                    )
                    nc.tensor.matmul(
                        out=gp[:, :w], lhsT=whh_bf[:, g * H : (g + 1) * H],
                        rhs=hbf[:, s0 : s0 + w], start=False, stop=True,
                    )
                    gs = gpool.tile([128, BANK], f32, tag=f"gs{g}")
                    nc.scalar.activation(
                        out=gs[:, :w], in_=gp[:, :w], func=AF.Sigmoid,
                        bias=brz_sb[:, g : g + 1],
                    )
                    gsb.append(gs)
                pni = ps.tile([128, BANK], f32, tag="ni", name=f"ni{tag}")
                nc.tensor.matmul(
                    out=pni[:, :w], lhsT=wih_bf[:, 2 * H : 3 * H],
                    rhs=aggbf[:, s0 : s0 + w], start=True, stop=True,
                )
                pnh = ps.tile([128, BANK], f32, tag="nh", name=f"nh{tag}")
                nc.tensor.matmul(
                    out=pnh[:, :w], lhsT=whh_bf[:, 2 * H : 3 * H],
                    rhs=hbf[:, s0 : s0 + w], start=True, stop=True,
                )
                hn = gpool.tile([128, BANK], f32, tag="hn")
                nc.vector.tensor_scalar_add(
                    out=hn[:, :w], in0=pnh[:, :w], scalar1=bhn_sb[:, :1]
                )
                nc.vector.tensor_tensor(
                    out=hn[:, :w], in0=hn[:, :w], in1=gsb[0][:, :w], op=OP.mult
                )
                ni = gpool.tile([128, BANK], f32, tag="ni_sb")
                nc.vector.tensor_scalar_add(
                    out=ni[:, :w], in0=pni[:, :w], scalar1=bin_sb[:, :1]
                )
                nc.vector.tensor_tensor(
                    out=ni[:, :w], in0=ni[:, :w], in1=hn[:, :w], op=OP.add
                )
                nc.scalar.activation(out=ni[:, :w], in_=ni[:, :w], func=AF.Tanh)
                d_t = gpool.tile([128, BANK], f32, tag="d_t")
                nc.vector.tensor_tensor(
                    out=d_t[:, :w], in0=hT[:, s0 : s0 + w], in1=ni[:, :w],
                    op=OP.subtract,
                )
                nc.vector.tensor_tensor(
                    out=d_t[:, :w], in0=d_t[:, :w], in1=gsb[1][:, :w], op=OP.mult
                )
                nc.vector.tensor_tensor(
                    out=hT[:, s0 : s0 + w], in0=d_t[:, :w], in1=ni[:, :w],
                    op=OP.add,
                )
                norm_strip(tag, s0, w)
                s0 += w

        # ---------------- encoder
        for s0 in range(0, npc_pad, BANK):
            w = min(BANK, npc_pad - s0)
            xst = rpool.tile([3, BANK], f32, tag="xst")
            nc.sync.dma_start(out=xst[:, :w], in_=xT_d[:, s0 : s0 + w])
            h0 = ps.tile([128, BANK], f32, tag="g0", name=f"enc{s0}")
            nc.tensor.matmul(
                out=h0[:, :w], lhsT=w1T_sb[:], rhs=xst[:, :w],
                start=True, stop=True,
            )
            nc.scalar.activation(
                out=hT[:, s0 : s0 + w], in_=h0[:, :w], func=AF.Relu,
                bias=b1_sb[:, :1],
            )
            nc.vector.memset(hmax[:, s0 : s0 + w], -1e30)
            norm_strip(f"e{s0}", s0, w)
        for k in range(4):
            store_rows(0, k)

        DST = 10

        def decode_quarter(k):
            t0q, tnq = qrt0[k], qrt0[k] + qrt[k]
            for t0 in range(t0q, tnq, DST):
                tn = min(t0 + DST, tnq)
                orows = decpool.tile(
                    [128, DST, 128], f32, tag="odec", name=f"od{t0}"
                )
                for t in range(t0, tn):
                    o_ps = psb.tile([128, BANK], f32, tag="bcns", name=f"dec{t}")
                    nc.tensor.matmul(
                        out=o_ps[:, :128], lhsT=onesk1_bf[:1, :], rhs=b2_bf[:1, :],
                        start=True, stop=False,
                    )
                    nc.tensor.matmul(
                        out=o_ps[:, :128], lhsT=hmax[:, t * 128 : (t + 1) * 128],
                        rhs=w2_bf[:], start=False, stop=True,
                    )
                    if t % 2 == 0:
                        nc.scalar.activation(
                            out=orows[:, t - t0, :], in_=o_ps[:, :128],
                            func=AF.Copy,
                        )
                    else:
                        nc.vector.tensor_copy(
                            out=orows[:, t - t0, :], in_=o_ps[:, :128]
                        )
                dst = out_d[t0 * 128 : tn * 128, :].rearrange(
                    "(u p) f -> p u f", p=128
                )
                nc.sync.dma_start(out=dst, in_=orows[:, : tn - t0, :])

        # ------------- message-passing layers (software-pipelined quarters)
        STG = 16  # scatter tiles per staging flush (16*64 = 1024 cols)
        for l in range(L):
            state = {"emitted": 0}
            eh_tiles = []

            def emit_gather_upto(chunk, l=l, state=state, eh_tiles=eh_tiles):
                want = min(chunk // PIECE_CHUNKS + 7, npieces)
                while state["emitted"] < want:
                    p = state["emitted"]
                    c0 = p * PIECE_CHUNKS
                    cip = min(PIECE_CHUNKS, nchunk - c0)
                    nidx = cip * 128
                    eh_t = ehpool.tile(
                        [128, PIECE_CHUNKS, 128], bf16, tag="eh",
                        name=f"eh{l}_{p}",
                    )
                    nc.gpsimd.dma_gather(
                        eh_t[:, :cip, :],
                        hrows[l % 2][:],
                        idx_sb[:, c0 * 8 : (c0 + cip) * 8],
                        nidx,
                        nidx,
                        128,
                    )
                    eh_tiles.append(eh_t)
                    state["emitted"] += 1

            def flush_store(k, qv, acc, ntiles, l=l):
                pd = partQ[k][l % 2]
                nc.sync.dma_start(
                    out=pd[qv * 128 : (qv + 1) * 128, : ntiles * TW],
                    in_=acc[:, : ntiles * TW],
                )

            acc = None
            acc_info = None
            acc_tiles = 0
            n_evac = 0
            cur_q = 0
            for g in range(n_gt):
                qv, tv = gt_list[g]
                if acc is not None and acc_info[1] != qv:
                    flush_store(*acc_info, acc, acc_tiles)
                    acc = None
                if acc is None:
                    acc = stpool.tile(
                        [128, max(qcols)], bf16, tag="acc", name=f"acc{l}_{g}"
                    )
                    acc_info = (cur_q, qv)
                    acc_tiles = 0
                sl = slice(acc_tiles * TW, acc_tiles * TW + TW)
                if not segs[g]:
                    nc.vector.memset(acc[:, sl], 0.0)
                else:
                    emit_gather_upto(segs[g][-1][0])
                    pst = ps_sc.tile([128, TW], f32, tag="sc", name=f"sc{l}_{g}")
                    ns_g = len(segs[g])
                    for si, (c, sid) in enumerate(segs[g]):
                        eh_t = eh_tiles[c // PIECE_CHUNKS]
                        cc = c % PIECE_CHUNKS
                        nc.tensor.matmul(
                            out=pst[:, :],
                            lhsT=eh_t[:, cc, :],
                            rhs=D_sb[:, sid * TW : (sid + 1) * TW],
                            start=(si == 0),
                            stop=(si == ns_g - 1),
                        )
                    if n_evac % 2 == 0:
                        nc.scalar.activation(
                            out=acc[:, sl], in_=pst[:, :], func=AF.Copy
                        )
                    else:
                        nc.vector.tensor_copy(out=acc[:, sl], in_=pst[:, :])
                    n_evac += 1
                acc_tiles += 1
                if g == qg0[cur_q + 1] - 1:  # group complete
                    flush_store(*acc_info, acc, acc_tiles)
                    acc = None
                    nc.gpsimd.collective_compute(
                        "ReduceScatter",
                        OP.add,
                        replica_groups=groups,
                        ins=[partQ[cur_q][l % 2][:]],
                        outs=[aggQ[cur_q][l % 2][:]],
                    )
                    # pipeline: previous quarter's GRU runs during the next
                    # quarter's scatter
                    if cur_q >= 1:
                        gru_quarter(l, cur_q - 1)
                        if l < L - 1:
                            store_rows(l + 1, cur_q - 1)
                        else:
                            decode_quarter(cur_q - 1)
                    cur_q += 1
            gru_quarter(l, 3)
            if l < L - 1:
                store_rows(l + 1, 3)

        stack.close()

    nc.compile()
    return nc


# ---------------------------------------------------------------- entry points
def _prep(inputs):
    x = np.asarray(inputs["x"], np.float32)
    edge_src = np.asarray(inputs["edge_src"], np.int64)
    edge_dst = np.asarray(inputs["edge_dst"], np.int64)
    n_nodes = x.shape[0]
    meta = _plan(edge_src, edge_dst, n_nodes)
    npc, npc_pad = meta["npc"], meta["npc_pad"]

    W1 = np.asarray(inputs["W1"], np.float32)
    b1 = np.asarray(inputs["b1"], np.float32)
    W_ih = np.asarray(inputs["W_ih"], np.float32)
    b_ih = np.asarray(inputs["b_ih"], np.float32)
    W_hh = np.asarray(inputs["W_hh"], np.float32)
    b_hh = np.asarray(inputs["b_hh"], np.float32)
    W2 = np.asarray(inputs["W2"], np.float32)
    b2 = np.asarray(inputs["b2"], np.float32)

    brz = np.stack(
        [b_ih[:H] + b_hh[:H], b_ih[H : 2 * H] + b_hh[H : 2 * H]], axis=1
    )
    shared = dict(
        w1T=np.ascontiguousarray(W1.T),
        b1=np.ascontiguousarray(b1[:, None]),
        wihT=np.ascontiguousarray(W_ih.T),
        whhT=np.ascontiguousarray(W_hh.T),
        brz=np.ascontiguousarray(brz),
        bin=np.ascontiguousarray(b_ih[2 * H :][:, None]),
        bhn=np.ascontiguousarray(b_hh[2 * H :][:, None]),
        w2T=np.ascontiguousarray(W2.T),
        b2=np.ascontiguousarray(b2[None, :]),
    )
    in_maps = []
    for r in range(CORES):
        xr = np.zeros((3, npc_pad), np.float32)
        xr[:, :npc] = x[r * npc : (r + 1) * npc].T
        in_maps.append(
            dict(
                xT=xr,
                idx=meta["cores"][r]["idx_w"],
                D=meta["cores"][r]["D"],
                **shared,
            )
        )
    return meta, in_maps


def _assemble(meta, results, n_nodes):
    npc = meta["npc"]
    out = np.empty((n_nodes, 128), np.float32)
    for r in range(CORES):
        out[r * npc : (r + 1) * npc] = results[r]["out"][:npc]
    return out


@functools.lru_cache(maxsize=1)
def _get_compiled(key):
    meta, in_maps = _PENDING[key]
    nc = _build(meta)
    return nc, meta, in_maps


_PENDING = {}


def kernel(**inputs):
    x = np.asarray(inputs["x"])
    n_nodes = x.shape[0]
    meta, in_maps = _prep(inputs)
    key = hash(
        (
            n_nodes,
            np.asarray(inputs["edge_src"]).tobytes(),
            np.asarray(inputs["edge_dst"]).tobytes(),
        )
    )
    _PENDING[key] = (meta, in_maps)
    nc, meta, _ = _get_compiled(key)

    from concourse.bass_utils import run_bass_kernel_spmd

    trace = bool(int(os.environ.get("KERNEL_TRACE", "0")))
    res = run_bass_kernel_spmd(
        nc, in_maps, core_ids=list(range(CORES)), trace=trace
    )
    kernel.last_results = res
    return _assemble(meta, res.results, n_nodes)
